# revision 26
# baseline (speedup 1.0000x reference)
"""Trainium2 Bass kernel for DirectContractedVoxGO alpha/weights/alphainv_last.

Math (per sample i in ray r):
    s_i     = softplus(density_i + shift) * interval
    alpha_i = 1 - exp(-s_i)
    T_i     = prod_{j<i in r} exp(-s_j)          (exclusive transmittance)
    w_i     = T_i * alpha_i
    alphainv_last_r = prod_{j in r} exp(-s_j) = T_excl[last] - w[last]

Device formulation (product form, no log-space pass needed):
    q = exp(-interval * softplus(density + shift))       (= 1 - alpha)
    T_excl_i = prod_{j<i in slot} q_j   -- one TensorTensorScan:
        state = d0_i * state + d1_i
        d0_i  = q_{i-1} (a one-column-shifted view; 0 at slot starts)
        d1_i  = 1 at slot starts else 0 (a compile-time-constant tile)
    alpha = 1 - q,  w = T_excl * alpha,  ainv = T_excl[end] - w[end]
    softplus(z) = ln(exp(z) + 1): Exp and Ln share one ACT table.

Layout: rays are length-sorted and packed into fixed-length slots, one ray
per slot, padded with density=-60 (=> q == 1.0f exactly, alpha = w = 0, so
padding is inert). Tile group t (same shape on every core - SPMD) gets the
next 128*S_SLOTS*8 rays by descending length and its own slot length
L_pads[t], so padding is only a few percent. Slot boundaries are
compile-time constants, hence the single-scan segmented product above.
"""

import numpy as np

import concourse.bacc as bacc
import concourse.mybir as mybir
import concourse.tile as tile
from concourse.bass_utils import run_bass_kernel_spmd

N_CORES = 8
P = 128
S_SLOTS = 4          # ray slots per partition per tile
PAD_DENSITY = -60.0  # softplus(-60+shift) == 0.0f -> q == 1.0f exactly

_prog_cache = {}
_last_results = None  # BassKernelResults of the most recent run (for harnesses)


def _build_program(L_pads, shift, interval, reps=1, s=None, alpha_act=False,
                   io_bufs=2, mid_bufs=2, probe=None, alpha_eng="dve",
                   w_eng="dve", small_eng="dve", formulation="expln",
                   wdma="sync", staggered=0):
    """One SPMD program; tile t is [P, s * L_pads[t]].

    reps>1 wraps the whole computation in a device-side loop (bench.py only).
    """
    if s is None:
        s = S_SLOTS
    plan = tuple(
        (int(x[0]), int(x[1])) if isinstance(x, (tuple, list)) else (s, int(x))
        for x in L_pads
    )
    if alpha_act:
        alpha_eng = "act"
    if formulation == "sig":
        assert interval == 0.5, "sig formulation is specialized to interval=0.5"
    key = (plan, shift, interval, reps, io_bufs, mid_bufs, probe,
           alpha_eng, w_eng, small_eng, formulation, wdma, staggered)
    if key in _prog_cache:
        return _prog_cache[key]

    f32 = mybir.dt.float32
    n_tiles = len(plan)
    n_slots_pp = sum(st for st, _ in plan)  # slots per partition
    FS = [st * lp for st, lp in plan]
    offs = np.concatenate([[0], np.cumsum([P * f for f in FS])]).astype(int)
    TOT = int(offs[-1])
    FS_max = max(FS)

    nc = bacc.Bacc("TRN2", target_bir_lowering=False, debug=False)
    dpack = nc.dram_tensor("dpack", [TOT], f32, kind="ExternalInput").ap()
    apack = nc.dram_tensor("apack", [TOT], f32, kind="ExternalOutput").ap()
    wpack = nc.dram_tensor("wpack", [TOT], f32, kind="ExternalOutput").ap()
    ainv = nc.dram_tensor(
        "ainv", [P, n_slots_pp], f32, kind="ExternalOutput"
    ).ap()

    def dview(t_ap, t):
        return t_ap[int(offs[t]) : int(offs[t + 1])].rearrange(
            "(p f) -> p f", f=FS[t]
        )

    with tile.TileContext(nc) as tc:
        with (
            tc.tile_pool(name="io", bufs=io_bufs) as io,
            tc.tile_pool(name="mid", bufs=mid_bufs) as mid,
            tc.tile_pool(name="aux", bufs=1) as aux,
        ):
            ainv_sb = aux.tile([P, n_slots_pp], f32, tag="ainv_sb")
            if probe == "dma":
                nc.vector.memset(ainv_sb[:], 1.0)
            shift_sb = aux.tile([P, 1], f32, tag="shift_sb")
            nc.vector.memset(shift_sb[:], shift)
            # d1 constants: 1.0 at slot-start columns, else 0. One per
            # distinct L_pad (tiles sharing L_pad share the constant).
            d1c = {}
            for st, lp in sorted(set(plan)):
                buf = aux.tile(
                    [P, st * lp], f32, tag=f"d1c_{st}_{lp}", name=f"d1c_{st}_{lp}"
                )
                nc.vector.memset(buf[:], 0.0)
                nc.vector.memset(
                    buf[:].rearrange("p (a b) -> p a b", b=lp)[:, :, 0:1], 1.0
                )
                d1c[(st, lp)] = buf

            import contextlib

            loop_ctx = (
                tc.For_i(0, reps, 1, staggered_reset=bool(staggered))
                if reps > 1
                else contextlib.nullcontext()
            )
            with loop_ctx:
                if probe == "dma":
                    # bandwidth probe: same bytes, no compute
                    for t in range(n_tiles):
                        din = io.tile(
                            [P, FS_max], f32, tag="din", name="din"
                        )[:, : FS[t]]
                        nc.sync.dma_start(din, dview(dpack, t))
                        nc.sync.dma_start(dview(apack, t), din)
                        nc.sync.dma_start(dview(wpack, t), din)
                elif formulation == "sig":
                    _emit_tiles_sig(
                        nc, plan, FS, FS_max, shift,
                        dview, dpack, apack, wpack,
                        io, mid, d1c, ainv_sb,
                        alpha_eng, small_eng, wdma,
                    )
                else:
                    _emit_tiles(
                        nc, plan, FS, FS_max, interval,
                        dview, dpack, apack, wpack,
                        io, mid, d1c, ainv_sb, shift_sb,
                        alpha_eng, w_eng, small_eng, probe,
                    )
            nc.sync.dma_start(ainv[:, :], ainv_sb[:])

    nc.compile()
    _prog_cache[key] = nc
    return nc


def _emit_tiles(
    nc, plan, FS, FS_max, interval,
    dview, dpack, apack, wpack,
    io, mid, d1c, ainv_sb, shift_sb,
    alpha_eng="dve", w_eng="dve", small_eng="dve", probe=None,
):
    f32 = mybir.dt.float32
    AF = mybir.ActivationFunctionType
    OP = mybir.AluOpType
    eng = {"dve": nc.vector, "pool": nc.gpsimd}
    slot0 = 0
    for t, (st, lp) in enumerate(plan):
        fs = FS[t]
        din = io.tile([P, FS_max], f32, tag="din", name="din")[:, :fs]
        nc.sync.dma_start(din, dview(dpack, t))

        # softplus(z) = ln(exp(z) + 1); Exp and Ln share one ACT table.
        u = mid.tile([P, FS_max], f32, tag="u", bufs=1, name="u")[:, :fs]
        nc.scalar.activation(u, din, AF.Exp, bias=shift_sb[:], scale=1.0)
        sp = mid.tile([P, FS_max], f32, tag="sp", bufs=1, name="sp")[:, :fs]
        nc.scalar.activation(sp, u, AF.Ln, bias=1.0, scale=1.0)
        # q lands in cols [1, fs] of qbuf; col 0 stays 0 so the shifted
        # view qbuf[:, 0:fs] reads q_{i-1} with q_{-1} = 0.
        qbuf = mid.tile([P, FS_max + 1], f32, tag="qbuf", name="qbuf")
        q = qbuf[:, 1 : fs + 1]
        nc.scalar.activation(q, sp, AF.Exp, scale=-interval)

        if probe == "actx":  # +2 value-preserving ACT passes (marginal-cost probe)
            qa = mid.tile([P, FS_max], f32, tag="qa", bufs=1, name="qa")[:, :fs]
            nc.scalar.activation(qa, q, AF.Identity)
            qb = mid.tile([P, FS_max], f32, tag="qb", bufs=1, name="qb")[:, :fs]
            nc.scalar.activation(qb, qa, AF.Identity)
            alpha_src = qb
        else:
            alpha_src = q

        al = io.tile([P, FS_max], f32, tag="al", name="al")[:, :fs]
        if alpha_eng == "act":
            nc.scalar.activation(al, alpha_src, AF.Identity, bias=1.0, scale=-1.0)
        else:
            eng[alpha_eng].tensor_scalar(al, alpha_src, -1.0, 1.0, OP.mult, OP.add)
        nc.sync.dma_start(dview(apack, t), al)

        # zero d0 at slot starts: qbuf cols {0, lp, 2lp, ...}
        eng[small_eng].memset(
            qbuf[:, 0:fs].rearrange("p (a b) -> p a b", b=lp)[:, :, 0:1], 0.0
        )

        T = mid.tile([P, FS_max], f32, tag="T", name="T")[:, :fs]
        nc.vector.tensor_tensor_scan(
            T, qbuf[:, 0:fs], d1c[(st, lp)][:], 0.0, OP.mult, OP.add
        )

        if probe == "dvex":  # +2 value-preserving DVE passes (marginal-cost probe)
            Ta = mid.tile([P, FS_max], f32, tag="Ta", bufs=1, name="Ta")[:, :fs]
            nc.vector.tensor_scalar(Ta, T, 1.0, 0.0, OP.mult, OP.add)
            Tb = mid.tile([P, FS_max], f32, tag="Tb", bufs=1, name="Tb")[:, :fs]
            nc.vector.tensor_scalar(Tb, Ta, 1.0, 0.0, OP.mult, OP.add)
            w_src = Tb
        else:
            w_src = T

        w = io.tile([P, FS_max], f32, tag="w", name="w")[:, :fs]
        eng[w_eng].tensor_tensor(w, w_src, al, OP.mult)
        nc.sync.dma_start(dview(wpack, t), w)

        # alphainv_last = T_incl[end] = T_excl[end] - w[end], strided views
        T3 = T.rearrange("p (a b) -> p a b", b=lp)
        w3 = w.rearrange("p (a b) -> p a b", b=lp)
        eng[small_eng].tensor_tensor(
            ainv_sb[:, slot0 : slot0 + st],
            T3[:, :, lp - 1 : lp],
            w3[:, :, lp - 1 : lp],
            OP.subtract,
        )
        slot0 += st


def _emit_tiles_sig(
    nc, plan, FS, FS_max, shift,
    dview, dpack, apack, wpack,
    io, mid, d1c, ainv_sb,
    alpha_eng="pool", small_eng="dve", wdma="sync",
):
    """interval=0.5 specialization: q = sqrt(sigmoid(-(d+shift))), two ACT
    passes with one table load each (all sigmoids, then all sqrts).
    w = T_excl[i] - T_excl[i+1] (difference of consecutive transmittances),
    so w does not depend on alpha; alpha can run on GPSIMD off the critical
    path. ainv = T_excl[end] * q[end]."""
    f32 = mybir.dt.float32
    AF = mybir.ActivationFunctionType
    OP = mybir.AluOpType
    eng = {"dve": nc.vector, "pool": nc.gpsimd, "act": nc.scalar}
    negshift_sb = mid.tile([P, 1], f32, tag="negshift_sb", bufs=1)
    nc.vector.memset(negshift_sb[:], -shift)

    n_tiles = len(plan)
    g_tiles = []
    for t, (st, lp) in enumerate(plan):
        fs = FS[t]
        din = io.tile([P, FS_max], f32, tag="din", name="din")[:, :fs]
        nc.sync.dma_start(din, dview(dpack, t))
        g = mid.tile(
            [P, FS_max], f32, tag=f"g{t}", bufs=1, name=f"g{t}"
        )[:, :fs]
        nc.scalar.activation(g, din, AF.Sigmoid, bias=negshift_sb[:], scale=-1.0)
        g_tiles.append(g)

    slot0 = 0
    for t, (st, lp) in enumerate(plan):
        fs = FS[t]
        qbuf = mid.tile([P, FS_max + 1], f32, tag="qbuf", bufs=3, name="qbuf")
        q = qbuf[:, 1 : fs + 1]
        nc.scalar.activation(q, g_tiles[t], AF.Sqrt)
        q3 = q.rearrange("p (a b) -> p a b", b=lp)

        al = io.tile([P, FS_max], f32, tag="al", name="al")[:, :fs]
        if alpha_eng == "act":
            nc.scalar.activation(al, q, AF.Identity, bias=1.0, scale=-1.0)
        else:
            eng[alpha_eng].tensor_scalar(al, q, -1.0, 1.0, OP.mult, OP.add)
        nc.sync.dma_start(dview(apack, t), al)

        # save q at slot ends (the shifted-view memset below zeroes them)
        qe = mid.tile([P, 16], f32, tag="qe", name="qe")[:, :st]
        eng[small_eng].tensor_copy(qe, q3[:, :, lp - 1 : lp])
        # zero d0 at slot starts: qbuf cols {0, lp, 2lp, ...} = q at slot ends
        eng[small_eng].memset(
            qbuf[:, 0:fs].rearrange("p (a b) -> p a b", b=lp)[:, :, 0:1], 0.0
        )

        T = mid.tile([P, FS_max], f32, tag="T", name="T")[:, :fs]
        nc.vector.tensor_tensor_scan(
            T, qbuf[:, 0:fs], d1c[(st, lp)][:], 0.0, OP.mult, OP.add
        )
        T3 = T.rearrange("p (a b) -> p a b", b=lp)

        # ainv = T_incl[end] = T_excl[end] * q[end]
        eng[small_eng].tensor_tensor(
            ainv_sb[:, slot0 : slot0 + st], T3[:, :, lp - 1 : lp], qe, OP.mult
        )

        w = io.tile([P, FS_max], f32, tag="w", name="w")[:, :fs]
        w3 = w.rearrange("p (a b) -> p a b", b=lp)
        nc.vector.tensor_tensor(
            w3[:, :, 0 : lp - 1],
            T3[:, :, 0 : lp - 1],
            T3[:, :, 1:lp],
            OP.subtract,
        )
        eng[small_eng].tensor_tensor(
            w3[:, :, lp - 1 : lp],
            T3[:, :, lp - 1 : lp],
            ainv_sb[:, slot0 : slot0 + st],
            OP.subtract,
        )
        wdma_eng = nc.scalar if wdma == "act" else nc.sync
        wdma_eng.dma_start(dview(wpack, t), w)
        slot0 += st


def _plan(ray_id, N_i, M):
    """Length-sorted slot plan. Returns per-tile ray-rank layout."""
    starts = np.searchsorted(ray_id, np.arange(N_i), side="left").astype(np.int64)
    counts = np.diff(np.append(starts, np.int64(M)))
    order = np.argsort(-counts, kind="stable")  # ray ids by descending length
    rays_per_tile = P * S_SLOTS * N_CORES
    n_tiles = N_i // rays_per_tile
    L_pads = []
    for t in range(n_tiles):
        grp = order[t * rays_per_tile : (t + 1) * rays_per_tile]
        L_pads.append(max(4, int(-(-int(counts[grp].max()) // 4) * 4)))
    return starts, counts, order, n_tiles, tuple(L_pads)


def _pack_inputs(density, ray_id, N_i, M):
    """Host-side shard/pack.

    Per-core layout: concat over tiles t of a [P, S_SLOTS*L_pads[t]] C-order
    block. Ray of rank (t*rays_per_tile + c*P*S_SLOTS + p*S_SLOTS + k) goes
    to core c, tile t, partition p, slot k.
    Returns (per_core_flat, gather_idx, valid_mask, meta).
    """
    starts, counts, order, n_tiles, L_pads = _plan(ray_id, N_i, M)
    spc = P * S_SLOTS  # slots (rays) per core per tile
    cores = []
    idxs = []   # per (t): [N_CORES, spc, L_pad] global sample index
    masks = []
    for t, lp in enumerate(L_pads):
        grp = order[t * spc * N_CORES : (t + 1) * spc * N_CORES]  # rank order
        st = starts[grp][:, None]
        cn = counts[grp][:, None]
        idx = st + np.arange(lp, dtype=np.int64)[None, :]
        mask = np.arange(lp)[None, :] < cn
        idxs.append(idx.reshape(N_CORES, spc, lp))
        masks.append(mask.reshape(N_CORES, spc, lp))
    per_core = []
    for c in range(N_CORES):
        blocks = []
        for t, lp in enumerate(L_pads):
            idx = idxs[t][c]
            mask = masks[t][c]
            blk = np.where(
                mask, density[np.minimum(idx, M - 1)], np.float32(PAD_DENSITY)
            ).astype(np.float32)
            # [spc, lp] -> [P, S_SLOTS*lp] (partition-major, slots contiguous)
            blocks.append(blk.reshape(P, S_SLOTS * lp).ravel())
        per_core.append(np.ascontiguousarray(np.concatenate(blocks)))
    meta = (starts, counts, order, n_tiles, L_pads)
    return per_core, idxs, masks, meta


def kernel(density, ray_id, shift, interval, N):
    density = np.ascontiguousarray(np.asarray(density), dtype=np.float32).ravel()
    ray_id = np.asarray(ray_id).ravel()
    shift_f = float(np.asarray(shift))
    interval_f = float(np.asarray(interval))
    N_i = int(np.asarray(N))
    M = density.shape[0]

    assert N_i % (N_CORES * P * S_SLOTS) == 0, N_i
    per_core, idxs, masks, meta = _pack_inputs(density, ray_id, N_i, M)
    starts, counts, order, n_tiles, L_pads = meta

    # interval=0.5 admits the cheaper 2-ACT-pass sqrt(sigmoid) formulation
    if interval_f == 0.5:
        nc = _build_program(L_pads, shift_f, interval_f, formulation="sig")
    else:
        nc = _build_program(L_pads, shift_f, interval_f)
    in_maps = [{"dpack": per_core[c]} for c in range(N_CORES)]
    res = run_bass_kernel_spmd(nc, in_maps, core_ids=list(range(N_CORES)))
    global _last_results
    _last_results = res

    alpha = np.empty(M, np.float32)
    weights = np.empty(M, np.float32)
    ainv = np.empty(N_i, np.float32)
    spc = P * S_SLOTS
    for c, r in enumerate(res.results):
        flat_a = r["apack"].ravel()
        flat_w = r["wpack"].ravel()
        off = 0
        for t, lp in enumerate(L_pads):
            n = P * S_SLOTS * lp
            idx = idxs[t][c]
            mask = masks[t][c]
            # [P, S_SLOTS*lp] -> [spc, lp] (inverse of pack reshape)
            a_blk = flat_a[off : off + n].reshape(spc, lp)
            w_blk = flat_w[off : off + n].reshape(spc, lp)
            alpha[idx[mask]] = a_blk[mask]
            weights[idx[mask]] = w_blk[mask]
            off += n
        # ainv_sb[p, t*S_SLOTS + k] -> rank t*spc*8 + c*spc + p*S_SLOTS + k
        av = r["ainv"].reshape(P, n_tiles, S_SLOTS)
        for t in range(n_tiles):
            ranks = order[
                t * spc * N_CORES + c * spc : t * spc * N_CORES + (c + 1) * spc
            ]
            ainv[ranks] = av[:, t, :].reshape(spc)
    return alpha, weights, ainv
